# revision 30
# baseline (speedup 1.0000x reference)
"""Trainium2 Bass kernel for nn_CausalSelfAttention_14980845928591.

Full-input contract: kernel(**inputs) takes the unsharded numpy inputs and
returns the full [B, T, C] output. Internally shards across 8 NeuronCores:
data-parallel over B (4 batches) x tensor-parallel over heads (2 groups of 8
heads). The output projection is a partial sum over head groups, reduced on
the host.

Single merged pipeline per core (no phase barriers):
  - projections (q/k/v/gate) per 512-wide T-block in fp32r (K=128 matmuls
    stream fastest in fp32r; weight load hides internally),
  - causal attention per i-tile in fp16 (K=64 / M=65 matmuls hit full PE
    stream rate in 16-bit; fp16 keeps quantization error ~1e-3),
  - output projection in fp32r.
Emission interleaves attention jb-steps (primary) with projection tile-jobs
(secondary) so the PE queue always holds independent work behind the
exp-dependent AV matmuls. exp runs on Act; sigmoid is computed as
0.5*(1+tanh(x/2)) so Act only ever needs the exp/tanh table (no reloads);
the 0.5 is folded into host-scaled w_proj and the +1 into a fused
scalar_tensor_tensor on DVE. Softmax row sums ride as a 65th V row; the
reciprocal uses the fast DVE approximation and is broadcast across
partitions via a DRAM round trip, all software-pipelined one i-tile behind
attention.
"""
import sys

sys.path.insert(0, "/opt/trn_rl_repo")

import numpy as np

import concourse.bass as bass
import concourse.mybir as mybir
import concourse.tile as tile
from concourse import bacc, bass_utils

# Problem shapes (hardcoded per contract).
B, T, C = 4, 2048, 1024
H, D = 16, 64
HL = 8            # heads per core
GC = HL * D       # 512: local q/k/v/gate/proj-row columns
P = 128
KC = C // P       # 8 contraction chunks
NTB = T // 512    # 4 T-blocks of 512
F32 = mybir.dt.float32
F32R = mybir.dt.float32r
F16 = mybir.dt.float16

_NC_CACHE = {}


def _build_nc():
    nc = bacc.Bacc("TRN2", target_bir_lowering=False, debug=False, num_devices=8)

    xT = nc.dram_tensor("xT", [C, T], F16, kind="ExternalInput")
    wqk = nc.dram_tensor("wqk", [C, 2 * GC], F16, kind="ExternalInput")
    wv = nc.dram_tensor("wv", [C, GC], F16, kind="ExternalInput")
    wg = nc.dram_tensor("wg", [C, GC], F16, kind="ExternalInput")
    bgh = nc.dram_tensor("bgh", [GC], F32, kind="ExternalInput")   # 0.5*b_gate
    wp = nc.dram_tensor("wp", [GC, C], F16, kind="ExternalInput")  # 0.5*w_proj
    outT = nc.dram_tensor("outT", [C, T], F16, kind="ExternalOutput")

    EXP = mybir.ActivationFunctionType.Exp
    TANH = mybir.ActivationFunctionType.Tanh

    with tile.TileContext(nc) as tc, \
         tc.tile_pool(name="pers", bufs=1) as pers, \
         tc.tile_pool(name="rot", bufs=2) as rot, \
         tc.tile_pool(name="psp", bufs=1, space="PSUM") as psp:

        # ---------- persistent tiles ----------
        kT = [pers.tile([P, T], F16, tag=f"kT{i}", name=f"kT{i}") for i in range(4)]
        vsb = [pers.tile([P, HL, 65], F16, tag=f"v{j}", name=f"v{j}")
               for j in range(T // P)]
        wqksb = [pers.tile([P, 2 * GC], F16, tag=f"wqk{k}", name=f"wqk{k}")
                 for k in range(KC)]
        wvsb = [pers.tile([P, GC], F16, tag=f"wv{k}", name=f"wv{k}")
                for k in range(KC)]
        wgsb = [pers.tile([P, GC], F16, tag=f"wg{k}", name=f"wg{k}")
                for k in range(KC)]
        wpsb = [pers.tile([P, C], F16, tag=f"wp{k}", name=f"wp{k}")
                for k in range(4)]
        umaskf = pers.tile([P, P], F32, tag="umaskf")
        umask = pers.tile([P, P], F16, tag="umask")
        zbias = pers.tile([P, 1], F32, tag="zbias")
        bgsb = pers.tile([P, 4], F32, tag="bg")
        ones64 = pers.tile([1, 64], F16, tag="ones64")
        warm = pers.tile([P, 256], F16, tag="warm")

        # constants
        nc.vector.memset(warm, 0.0)
        nc.vector.memset(ones64, 1.0)
        nc.gpsimd.memset(zbias, 0.0)
        nc.gpsimd.memset(umaskf, 1.0)
        # keep element where (col - row) >= 0 (upper triangular incl diag)
        nc.gpsimd.affine_select(
            out=umaskf, in_=umaskf, compare_op=mybir.AluOpType.is_ge,
            fill=0.0, base=0, pattern=[[1, P]], channel_multiplier=-1,
        )
        nc.vector.tensor_copy(out=umask, in_=umaskf)
        nc.sync.dma_start(out=bgsb, in_=bgh.rearrange("(m p) -> p m", m=4))

        state = {}

        # PE warmup: dummy matmuls spanning the initial DMA window so the
        # HAM clock gate flips to 8/8 before the first real matmul and the
        # projections never run cold.  Only depends on a vector memset, so
        # the chain starts within ~1us of kernel entry.
        wps = psp.tile([P, 512], F32, tag="psA", name="psA", bufs=2)
        for _ in range(28):
            nc.tensor.matmul(wps[:, 0:256], warm[:, 0:P], warm,
                             start=True, stop=True)

        # ---------- startup: wqk + x(tb0) interleaved ----------
        xt0 = []
        for k in range(KC):
            nc.sync.dma_start(out=wqksb[k], in_=wqk[k * P:(k + 1) * P, :])
            t_ = rot.tile([P, 512], F16, tag="xs", name="xs", bufs=12)
            nc.sync.dma_start(out=t_, in_=xT[k * P:(k + 1) * P, 0:512])
            xt0.append(t_)
        state[("xt", 0)] = xt0

        # ---------- job builders ----------
        def mk_xt_dma(tb):
            def job():
                xt = []
                for k in range(KC):
                    t_ = rot.tile([P, 512], F16, tag="xs", name="xs", bufs=12)
                    nc.sync.dma_start(
                        out=t_, in_=xT[k * P:(k + 1) * P, tb * 512:(tb + 1) * 512])
                    xt.append(t_)
                state[("xt", tb)] = xt
            return job

        def mk_wvg_dma():
            def job():
                for k in range(KC):
                    nc.sync.dma_start(out=wvsb[k], in_=wv[k * P:(k + 1) * P, :])
                    nc.sync.dma_start(out=wgsb[k], in_=wg[k * P:(k + 1) * P, :])
            return job

        def mk_wp_dma():
            def job():
                for k in range(4):
                    nc.sync.dma_start(out=wpsb[k], in_=wp[k * P:(k + 1) * P, :])
            return job

        def mk_qk(tb, m):
            def job():
                xt = state[("xt", tb)]
                tsl = slice(tb * 512, (tb + 1) * 512)
                ps = psp.tile([P, 512], F32, tag="psA", name="psA", bufs=2)
                for k in range(KC):
                    nc.tensor.matmul(ps, wqksb[k][:, m * P:(m + 1) * P], xt[k],
                                     start=(k == 0), stop=(k == KC - 1))
                if m < 4:
                    q_t = rot.tile([P, 512], F16, tag=f"q{m}", name=f"q{m}", bufs=2)
                    nc.vector.tensor_copy(out=q_t, in_=ps)
                    state[("q", tb, m)] = q_t
                else:
                    nc.vector.tensor_copy(out=kT[m - 4][:, tsl], in_=ps)
            return job

        def mk_gate(tb, m):
            def job():
                xt = state[("xt", tb)]
                ps = psp.tile([P, 512], F32, tag="psA", name="psA", bufs=2)
                for k in range(KC):
                    nc.tensor.matmul(ps, wgsb[k][:, m * P:(m + 1) * P], xt[k],
                                     start=(k == 0), stop=(k == KC - 1))
                g_t = rot.tile([P, 512], F16, tag=f"g{m}", name=f"g{m}", bufs=2)
                # tanh(0.5*x + 0.5*b); sigmoid = 0.5*(1+tanh(...)): the +1 is
                # fused into the gate multiply, the 0.5 into host-scaled wp.
                nc.scalar.activation(g_t, ps, TANH, bias=bgsb[:, m:m + 1],
                                     scale=0.5)
                state[("g", tb, m)] = g_t
            return job

        def mk_v(tb, mt):
            def job():
                xt = state[("xt", tb)]
                j = tb * 4 + mt
                ps = psp.tile([P, 512], F32, tag="psA", name="psA", bufs=2)
                for k in range(KC):
                    nc.tensor.matmul(ps, xt[k][:, mt * P:(mt + 1) * P], wvsb[k],
                                     start=(k == 0), stop=(k == KC - 1))
                nc.vector.tensor_copy(
                    out=vsb[j][:, :, 0:64],
                    in_=ps.rearrange("p (h d) -> p h d", h=HL))
                nc.vector.memset(vsb[j][:, :, 64:65], 1.0)
            return job

        def qk_jobs(tb):
            return [(mk_qk(tb, m), 4096) for m in range(8)]

        def v_jobs(tb, cost=4096):
            return [(mk_v(tb, mt), cost) for mt in range(4)]

        def gate_jobs(tb):
            return [(mk_gate(tb, m), 4096) for m in range(4)]

        # ---------- attention ----------
        def emit_av(it, p, jb, njb):
            ko = jb - 4 * it
            c0 = P * ko if ko > 0 else 0
            if jb % 2 == 1:
                pTp = state.pop(("pTp", it, p, jb // 2))
            else:
                pTp = state[("pTp", it, p, jb // 2)]
            off = (jb % 2) * 1024
            st, sp = (jb == 0), (jb == njb - 1)
            if st:
                state[("O2", it, p)] = psp.tile([65, 1024], F32, tag="O2",
                                                name="O2", bufs=1)
            O2 = state[("O2", it, p)]
            nc.tensor.matmul(O2[:, c0:512], vsb[jb][:, 2 * p, :],
                             pTp[:, off + c0:off + 512], start=st, stop=sp,
                             skip_group_check=True)
            nc.tensor.matmul(O2[:, 512 + c0:1024], vsb[jb][:, 2 * p + 1, :],
                             pTp[:, off + 512 + c0:off + 1024], start=st,
                             stop=sp, skip_group_check=True)

        def mk_jb(it, p, jb, njb):
            def job():
                ko = jb - 4 * it
                c0 = P * ko if ko > 0 else 0
                jsl = slice(jb * P, (jb + 1) * P)
                q_t = state[("q", it, p)]
                # s2/pT are PAIR tiles covering two jb halves; full (off-
                # diagonal) pairs get ONE batched exp, halving the Act
                # 352-cycle per-ACTIVATE overhead on the exp-bound chain.
                if jb % 2 == 0:
                    state[("s2", it, p)] = psp.tile([P, 2048], F32, tag="s2",
                                                    name="s2", bufs=1)
                    state[("pTp", it, p, jb // 2)] = rot.tile(
                        [P, 2048], F16, tag="pT", name="pT", bufs=2)
                s2p = state[("s2", it, p)]
                pTp = state[("pTp", it, p, jb // 2)]
                off = (jb % 2) * 1024
                nc.tensor.matmul(s2p[:, off + c0:off + 512], kT[p][0:64, jsl],
                                 q_t[0:64, c0:512], start=True, stop=True)
                nc.tensor.matmul(s2p[:, off + 512 + c0:off + 1024],
                                 kT[p][64:128, jsl],
                                 q_t[64:128, c0:512], start=True, stop=True)
                ko_odd = 2 * (jb // 2) + 1 - 4 * it
                if ko_odd <= 0:
                    # full pair: single exp over both halves at the odd jb
                    if jb % 2 == 1:
                        nc.scalar.activation(pTp, s2p, EXP, bias=zbias,
                                             scale=0.125)
                else:
                    # diagonal pair: per-jb exp past the causal skip
                    if ko > 0:
                        sv = s2p.rearrange("p (j h f) -> p j h f", j=2, h=2)[
                            :, jb % 2, :, c0:512]
                        pv = pTp.rearrange("p (j h f) -> p j h f", j=2, h=2)[
                            :, jb % 2, :, c0:512]
                        nc.scalar.activation(pv, sv, EXP, bias=zbias,
                                             scale=0.125)
                    else:
                        nc.scalar.activation(pTp[:, off:off + 1024],
                                             s2p[:, off:off + 1024],
                                             EXP, bias=zbias, scale=0.125)
                    if ko >= 0:
                        # causal mask on the diagonal 128x128 block of both
                        # heads via one gpsimd affine_select (keep where
                        # col-row >= 0); gpsimd is otherwise idle.
                        mv = pTp.rearrange("p (j h f) -> p j h f", j=2, h=2)[
                            :, jb % 2, :, c0:c0 + P]
                        nc.gpsimd.affine_select(
                            out=mv, in_=mv, compare_op=mybir.AluOpType.is_ge,
                            fill=0.0, base=0, pattern=[[0, 2], [1, P]],
                            channel_multiplier=-1,
                        )
                # AV trails QK by two jb-steps: by the time the AV pair
                # reaches the head of the PE FIFO its exp+mask inputs are
                # long done, so the PE never head-of-line blocks on Act.
                if jb > 1:
                    emit_av(it, p, jb - 2, njb)
            return job

        def mk_tail(it, p, njb):
            def job():
                if njb > 1:
                    emit_av(it, p, njb - 2, njb)
                emit_av(it, p, njb - 1, njb)
                state.pop(("s2", it, p), None)
                O2 = state.pop(("O2", it, p))
                o_t = rot.tile([P, 512], F16, tag=f"o{p}", name=f"o{p}", bufs=2)
                nc.vector.tensor_copy(out=o_t[0:64, :], in_=O2[0:64, 0:512])
                nc.vector.tensor_copy(out=o_t[64:128, :], in_=O2[0:64, 512:1024])
                # softmax denominators: row 64 of O2 -> f16 SBUF row;
                # partition-broadcast happens later via a K=1 ones-matmul in
                # mk_y_recip -- no DRAM round trip.  DVE, not Act: Act is the
                # scarce engine in the diagonal/tail region (exp backlog).
                row = rot.tile([1, 1024], F16, tag="srow", name="srow", bufs=4)
                nc.vector.tensor_copy(out=row, in_=O2[64:65, :])
                state[("srow", it, p)] = row
                state[("osb", it, p)] = o_t
            return job

        def nfree(it, jb):
            ko = jb - 4 * it
            return 512 - 128 * ko if ko > 0 else 512

        def attn_jobs(it):
            njb = 4 * it + 4
            jobs = []
            for p in range(4):
                for jb in range(njb):
                    c = nfree(it, jb) + (2 * nfree(it, jb - 2) if jb >= 2
                                         else 0)
                    jobs.append((mk_jb(it, p, jb, njb), c))
                jobs.append((mk_tail(it, p, njb),
                             2 * nfree(it, njb - 2) + 2 * nfree(it, njb - 1)))
            return jobs

        # ---------- normalization + output projection ----------

        def mk_y_recip(it, p):
            def job():
                row = state.pop(("srow", it, p))
                ps = psp.tile([P, 512], F32, tag="psA", name="psA", bufs=2)
                # broadcast sums across partitions: ones[1,64]^T @ row[1,512]
                nc.tensor.matmul(ps[0:64, :], ones64, row[0:1, 0:512],
                                 start=True, stop=True)
                nc.tensor.matmul(ps[64:128, :], ones64, row[0:1, 512:1024],
                                 start=True, stop=True)
                rb_t = rot.tile([P, 512], F32, tag=f"rb{p}", name=f"rb{p}",
                                bufs=1)
                nc.vector.reciprocal_approx_fast(out=rb_t, in_=ps)
                state[("rb", it, p)] = rb_t
            return job

        def mk_y_mul(it, p):
            def job():
                g_t = state.pop(("g", it, p))
                o_t = state.pop(("osb", it, p))
                rb_t = state.pop(("rb", it, p))
                tt = rot.tile([P, 512], F32, tag="tt", name="tt", bufs=2)
                # (1 + tanh) * O  -- the 0.5 of the sigmoid identity is folded
                # into wp (host-scaled), the row-sum recip comes via rb.
                nc.vector.scalar_tensor_tensor(
                    out=tt, in0=g_t, scalar=1.0, in1=o_t,
                    op0=mybir.AluOpType.add, op1=mybir.AluOpType.mult)
                y_t = rot.tile([P, 512], F16, tag=f"y{p}", name=f"y{p}", bufs=2)
                nc.vector.tensor_mul(y_t, tt, rb_t)
                state[("y", it, p)] = y_t
            return job

        def norm_jobs(it, ps=range(4)):
            jobs = []
            for p in ps:
                jobs.append((mk_y_recip(it, p), 1024))
                jobs.append((mk_y_mul(it, p), 0))
            return jobs

        def mk_proj(it, m):
            def job():
                ps = psp.tile([P, 512], F32, tag="psA", name="psA", bufs=2)
                for k in range(4):
                    nc.tensor.matmul(ps, wpsb[k][:, m * P:(m + 1) * P],
                                     state[("y", it, k)],
                                     start=(k == 0), stop=(k == 3))
                ob = rot.tile([P, 512], F16, tag="ob", name="ob",
                               bufs=2)
                if it == NTB - 1 and m % 2 == 0:
                    # alternate Act/DVE at the kernel tail so consecutive
                    # drains overlap instead of queueing on one engine
                    nc.scalar.activation(ob, ps,
                                         mybir.ActivationFunctionType.Copy,
                                         bias=0.0, scale=1.0)
                else:
                    nc.vector.tensor_copy(out=ob, in_=ps)
                nc.sync.dma_start(
                    out=outT[m * P:(m + 1) * P, it * 512:(it + 1) * 512],
                    in_=ob)
                if m == 7:
                    for k in range(4):
                        state.pop(("y", it, k))
            return job

        def proj_jobs(it):
            return [(mk_proj(it, m), 2048) for m in range(8)]

        # ---------- emission schedule ----------
        # Pace secondary jobs against primary by PE-work cost, not count, so
        # the cheap diagonal/tail stretch of each i-tile gets proportionally
        # more filler and the PE never starves there.
        def emit_interleaved(primary, secondary):
            ptot = sum(c for _, c in primary) or 1
            stot = sum(c for _, c in secondary) or 1
            si = 0
            pacc = 0.0
            sacc = 0.0
            for job, c in primary:
                job()
                pacc += c
                target = pacc / ptot
                while si < len(secondary) and \
                        (sacc + secondary[si][1] * 0.5) / stot <= target:
                    secondary[si][0]()
                    sacc += secondary[si][1]
                    si += 1
            while si < len(secondary):
                secondary[si][0]()
                si += 1

        # Only tb0's q/k projections gate the start of attention; v/gate
        # jobs become it0 filler.  gate(tb) is only consumed by norm(tb)
        # one i-tile later, so it shifts one span later too -- this drains
        # surplus PE filler from the early (PE-bound) spans.  gate(3) stays
        # in it2: its tanh would land on it3's critical Act chain.
        for job, _ in qk_jobs(0):
            job()
        for it in range(NTB):
            primary = attn_jobs(it)
            secondary = []
            if it == 0:
                secondary.append((mk_wvg_dma(), 0))
                secondary += v_jobs(0, cost=0)
            if it + 1 < NTB:
                secondary.append((mk_xt_dma(it + 1), 0))
            if it == 0:
                secondary.append((mk_wp_dma(), 0))
                secondary += gate_jobs(0)
            if it >= 1:
                secondary += norm_jobs(it - 1)
            if it in (1, 2):
                secondary += gate_jobs(it)
            if it == 2:
                secondary += gate_jobs(3)
            if it == 1:
                secondary += proj_jobs(0)
            if it == NTB - 1:
                # back-load the PE-heavy projection jobs into the final,
                # Act-bound attention segment to fill PE idle slots; the
                # final i-tile's own y-jobs go last so each lands right
                # after its p-group's drain
                secondary += proj_jobs(1) + proj_jobs(2)
                secondary += norm_jobs(3, ps=range(3))
            if it + 1 < NTB:
                secondary += qk_jobs(it + 1)
                secondary += v_jobs(it + 1)
            emit_interleaved(primary, secondary)
        for job, _ in norm_jobs(NTB - 1, ps=[3]) + proj_jobs(NTB - 1):
            job()

    nc.compile()
    return nc


def kernel(x, w_attn, w_proj, w_gate, b_gate):
    x = np.ascontiguousarray(np.asarray(x, dtype=np.float32))
    w_attn = np.asarray(w_attn, dtype=np.float32)
    w_proj = np.asarray(w_proj, dtype=np.float32)
    w_gate = np.asarray(w_gate, dtype=np.float32)
    b_gate = np.asarray(b_gate, dtype=np.float32)

    if "nc" not in _NC_CACHE:
        _NC_CACHE["nc"] = _build_nc()
    nc = _NC_CACHE["nc"]

    in_maps = []
    for c in range(8):
        b, g = c // 2, c % 2
        hsl = slice(g * GC, (g + 1) * GC)
        in_maps.append({
            "xT": np.ascontiguousarray(x[b].T.astype(np.float16)),
            "wqk": np.ascontiguousarray(
                np.concatenate([w_attn[:, hsl],
                                w_attn[:, C + g * GC:C + (g + 1) * GC]],
                               axis=1).astype(np.float16)),
            "wv": np.ascontiguousarray(
                w_attn[:, 2 * C + g * GC:2 * C + (g + 1) * GC].astype(np.float16)),
            "wg": np.ascontiguousarray(w_gate[:, hsl].astype(np.float16)),
            "bgh": np.ascontiguousarray(b_gate[hsl] * 0.5),
            "wp": np.ascontiguousarray((w_proj[hsl, :] * 0.5).astype(np.float16)),
        })

    res = bass_utils.run_bass_kernel_spmd(nc, in_maps, core_ids=list(range(8)))

    out = np.empty((B, T, C), dtype=np.float32)
    for b in range(B):
        acc = res.results[2 * b]["outT"].astype(np.float32)
        acc = acc + res.results[2 * b + 1]["outT"].astype(np.float32)
        out[b] = acc.T
    return out



# revision 33
# speedup vs baseline: 1.1027x; 1.1027x over previous
"""Trainium2 Bass kernel for nn_CausalSelfAttention_14980845928591.

Full-input contract: kernel(**inputs) takes the unsharded numpy inputs and
returns the full [B, T, C] output. Internally shards across 8 NeuronCores:
data-parallel over B (4 batches) x tensor-parallel over heads (2 groups of 8
heads). The output projection is a partial sum over head groups, reduced on
the host.

Single merged pipeline per core (no phase barriers):
  - projections (q/k/v/gate) per 512-wide T-block in fp32r (K=128 matmuls
    stream fastest in fp32r; weight load hides internally),
  - causal attention per i-tile in fp16 (K=64 / M=65 matmuls hit full PE
    stream rate in 16-bit; fp16 keeps quantization error ~1e-3),
  - output projection in fp32r.
Emission interleaves attention jb-steps (primary) with projection tile-jobs
(secondary) so the PE queue always holds independent work behind the
exp-dependent AV matmuls. exp runs on Act; sigmoid is computed as
0.5*(1+tanh(x/2)) so Act only ever needs the exp/tanh table (no reloads);
the 0.5 is folded into host-scaled w_proj and the +1 into a fused
scalar_tensor_tensor on DVE. Softmax row sums ride as a 65th V row; the
reciprocal uses the fast DVE approximation and is broadcast across
partitions via a DRAM round trip, all software-pipelined one i-tile behind
attention.
"""
import sys

sys.path.insert(0, "/opt/trn_rl_repo")

import numpy as np

import concourse.bass as bass
import concourse.mybir as mybir
import concourse.tile as tile
from concourse import bacc, bass_utils

# Problem shapes (hardcoded per contract).
B, T, C = 4, 2048, 1024
H, D = 16, 64
HL = 8            # heads per core
GC = HL * D       # 512: local q/k/v/gate/proj-row columns
P = 128
KC = C // P       # 8 contraction chunks
NTB = T // 512    # 4 T-blocks of 512
F32 = mybir.dt.float32
F32R = mybir.dt.float32r
F16 = mybir.dt.float16

_NC_CACHE = {}


def _build_nc():
    nc = bacc.Bacc("TRN2", target_bir_lowering=False, debug=False, num_devices=8)

    xT = nc.dram_tensor("xT", [C, T], F16, kind="ExternalInput")
    wqk = nc.dram_tensor("wqk", [C, 2 * GC], F16, kind="ExternalInput")
    wv = nc.dram_tensor("wv", [C, GC], F16, kind="ExternalInput")
    wg = nc.dram_tensor("wg", [C, GC], F16, kind="ExternalInput")
    bgh = nc.dram_tensor("bgh", [GC], F32, kind="ExternalInput")   # 0.5*b_gate
    wp = nc.dram_tensor("wp", [GC, C], F16, kind="ExternalInput")  # 0.5*w_proj
    outT = nc.dram_tensor("outT", [C, T], F16, kind="ExternalOutput")

    EXP = mybir.ActivationFunctionType.Exp
    TANH = mybir.ActivationFunctionType.Tanh

    with tile.TileContext(nc) as tc, \
         tc.tile_pool(name="pers", bufs=1) as pers, \
         tc.tile_pool(name="rot", bufs=2) as rot, \
         tc.tile_pool(name="psp", bufs=1, space="PSUM") as psp:

        # ---------- persistent tiles ----------
        kT = [pers.tile([P, T], F16, tag=f"kT{i}", name=f"kT{i}") for i in range(4)]
        vsb = [pers.tile([P, HL, 65], F16, tag=f"v{j}", name=f"v{j}")
               for j in range(T // P)]
        wqksb = [pers.tile([P, 2 * GC], F16, tag=f"wqk{k}", name=f"wqk{k}")
                 for k in range(KC)]
        wvsb = [pers.tile([P, GC], F16, tag=f"wv{k}", name=f"wv{k}")
                for k in range(KC)]
        wgsb = [pers.tile([P, GC], F16, tag=f"wg{k}", name=f"wg{k}")
                for k in range(KC)]
        wpsb = [pers.tile([P, C], F16, tag=f"wp{k}", name=f"wp{k}")
                for k in range(4)]
        umaskf = pers.tile([P, P], F32, tag="umaskf")
        umask = pers.tile([P, P], F16, tag="umask")
        zbias = pers.tile([P, 1], F32, tag="zbias")
        bgsb = pers.tile([P, 4], F32, tag="bg")
        ones64 = pers.tile([1, 64], F16, tag="ones64")
        warm = pers.tile([P, 256], F16, tag="warm")

        # constants
        nc.vector.memset(warm, 0.0)
        nc.vector.memset(ones64, 1.0)
        nc.gpsimd.memset(zbias, 0.0)
        nc.gpsimd.memset(umaskf, 1.0)
        # keep element where (col - row) >= 0 (upper triangular incl diag)
        nc.gpsimd.affine_select(
            out=umaskf, in_=umaskf, compare_op=mybir.AluOpType.is_ge,
            fill=0.0, base=0, pattern=[[1, P]], channel_multiplier=-1,
        )
        nc.vector.tensor_copy(out=umask, in_=umaskf)
        nc.sync.dma_start(out=bgsb, in_=bgh.rearrange("(m p) -> p m", m=4))

        state = {}

        # PE warmup: dummy matmuls spanning the initial DMA window so the
        # HAM clock gate flips to 8/8 before the first real matmul and the
        # projections never run cold.  Only depends on a vector memset, so
        # the chain starts within ~1us of kernel entry.
        wps = psp.tile([P, 512], F32, tag="psA", name="psA", bufs=2)
        for _ in range(44):
            nc.tensor.matmul(wps[:, 0:256], warm[:, 0:P], warm,
                             start=True, stop=True)

        # ---------- startup: wqk + x(tb0) interleaved ----------
        xt0 = []
        for k in range(KC):
            nc.sync.dma_start(out=wqksb[k], in_=wqk[k * P:(k + 1) * P, :])
            t_ = rot.tile([P, 512], F16, tag="xs", name="xs", bufs=12)
            nc.sync.dma_start(out=t_, in_=xT[k * P:(k + 1) * P, 0:512])
            xt0.append(t_)
        state[("xt", 0)] = xt0

        # ---------- job builders ----------
        def mk_xt_dma(tb):
            def job():
                xt = []
                for k in range(KC):
                    t_ = rot.tile([P, 512], F16, tag="xs", name="xs", bufs=12)
                    nc.sync.dma_start(
                        out=t_, in_=xT[k * P:(k + 1) * P, tb * 512:(tb + 1) * 512])
                    xt.append(t_)
                state[("xt", tb)] = xt
            return job

        def mk_wvg_dma():
            def job():
                for k in range(KC):
                    nc.sync.dma_start(out=wvsb[k], in_=wv[k * P:(k + 1) * P, :])
                    nc.sync.dma_start(out=wgsb[k], in_=wg[k * P:(k + 1) * P, :])
            return job

        def mk_wp_dma():
            def job():
                for k in range(4):
                    nc.sync.dma_start(out=wpsb[k], in_=wp[k * P:(k + 1) * P, :])
            return job

        def mk_qk(tb, m):
            def job():
                xt = state[("xt", tb)]
                tsl = slice(tb * 512, (tb + 1) * 512)
                ps = psp.tile([P, 512], F32, tag="psA", name="psA", bufs=2)
                for k in range(KC):
                    nc.tensor.matmul(ps, wqksb[k][:, m * P:(m + 1) * P], xt[k],
                                     start=(k == 0), stop=(k == KC - 1))
                if m < 4:
                    q_t = rot.tile([P, 512], F16, tag=f"q{m}", name=f"q{m}", bufs=2)
                    nc.vector.tensor_copy(out=q_t, in_=ps)
                    state[("q", tb, m)] = q_t
                else:
                    nc.vector.tensor_copy(out=kT[m - 4][:, tsl], in_=ps)
            return job

        def mk_gate(tb, m):
            def job():
                xt = state[("xt", tb)]
                ps = psp.tile([P, 512], F32, tag="psA", name="psA", bufs=2)
                for k in range(KC):
                    nc.tensor.matmul(ps, wgsb[k][:, m * P:(m + 1) * P], xt[k],
                                     start=(k == 0), stop=(k == KC - 1))
                g_t = rot.tile([P, 512], F16, tag=f"g{m}", name=f"g{m}", bufs=2)
                # tanh(0.5*x + 0.5*b); sigmoid = 0.5*(1+tanh(...)): the +1 is
                # fused into the gate multiply, the 0.5 into host-scaled wp.
                nc.scalar.activation(g_t, ps, TANH, bias=bgsb[:, m:m + 1],
                                     scale=0.5)
                state[("g", tb, m)] = g_t
            return job

        def mk_v(tb, mt):
            def job():
                xt = state[("xt", tb)]
                j = tb * 4 + mt
                ps = psp.tile([P, 512], F32, tag="psA", name="psA", bufs=2)
                for k in range(KC):
                    nc.tensor.matmul(ps, xt[k][:, mt * P:(mt + 1) * P], wvsb[k],
                                     start=(k == 0), stop=(k == KC - 1))
                nc.vector.tensor_copy(
                    out=vsb[j][:, :, 0:64],
                    in_=ps.rearrange("p (h d) -> p h d", h=HL))
                nc.vector.memset(vsb[j][:, :, 64:65], 1.0)
            return job

        def qk_jobs(tb):
            return [(mk_qk(tb, m), 4096) for m in range(8)]

        def v_jobs(tb, cost=4096):
            return [(mk_v(tb, mt), cost) for mt in range(4)]

        def gate_jobs(tb):
            return [(mk_gate(tb, m), 4096) for m in range(4)]

        # ---------- attention ----------
        def emit_av(it, p, jb, njb):
            ko = jb - 4 * it
            c0 = P * ko if ko > 0 else 0
            pT = state.pop(("pT", it, p, jb))
            st, sp = (jb == 0), (jb == njb - 1)
            if st:
                state[("O2", it, p)] = psp.tile([65, 1024], F32, tag="O2",
                                                name="O2", bufs=1)
            O2 = state[("O2", it, p)]
            nc.tensor.matmul(O2[:, c0:512], vsb[jb][:, 2 * p, :],
                             pT[:, c0:512], start=st, stop=sp,
                             skip_group_check=True)
            nc.tensor.matmul(O2[:, 512 + c0:1024], vsb[jb][:, 2 * p + 1, :],
                             pT[:, 512 + c0:1024], start=st, stop=sp,
                             skip_group_check=True)

        def mk_jb(it, p, jb, njb):
            def job():
                ko = jb - 4 * it
                c0 = P * ko if ko > 0 else 0
                jsl = slice(jb * P, (jb + 1) * P)
                q_t = state[("q", it, p)]
                s2 = psp.tile([P, 1024], F32, tag="s2", name="s2", bufs=2)
                nc.tensor.matmul(s2[:, c0:512], kT[p][0:64, jsl],
                                 q_t[0:64, c0:512], start=True, stop=True)
                nc.tensor.matmul(s2[:, 512 + c0:1024], kT[p][64:128, jsl],
                                 q_t[64:128, c0:512], start=True, stop=True)
                pT = rot.tile([P, 1024], F16, tag="pT", name="pT", bufs=3)
                if ko > 0:
                    # one strided activation covers both heads' live columns
                    sv = s2.rearrange("p (h f) -> p h f", h=2)[:, :, c0:512]
                    pv = pT.rearrange("p (h f) -> p h f", h=2)[:, :, c0:512]
                    nc.scalar.activation(pv, sv, EXP, bias=zbias, scale=0.125)
                else:
                    nc.scalar.activation(pT, s2, EXP, bias=zbias, scale=0.125)
                if ko >= 0:
                    # causal mask on the diagonal 128x128 block of both heads
                    # via one gpsimd affine_select (keep where col-row >= 0);
                    # gpsimd is otherwise idle, freeing DVE for drains/casts.
                    mv = pT.rearrange("p (h f) -> p h f", h=2)[:, :, c0:c0 + P]
                    nc.gpsimd.affine_select(
                        out=mv, in_=mv, compare_op=mybir.AluOpType.is_ge,
                        fill=0.0, base=0, pattern=[[0, 2], [1, P]],
                        channel_multiplier=-1,
                    )
                state[("pT", it, p, jb)] = pT
                # AV trails QK by two jb-steps: by the time the AV pair
                # reaches the head of the PE FIFO its exp+mask inputs are
                # long done, so the PE never head-of-line blocks on Act.
                if jb > 1:
                    emit_av(it, p, jb - 2, njb)
            return job

        def mk_tail(it, p, njb):
            def job():
                if njb > 1:
                    emit_av(it, p, njb - 2, njb)
                emit_av(it, p, njb - 1, njb)
                O2 = state.pop(("O2", it, p))
                o_t = rot.tile([P, 512], F16, tag=f"o{p}", name=f"o{p}", bufs=2)
                nc.vector.tensor_copy(out=o_t[0:64, :], in_=O2[0:64, 0:512])
                nc.vector.tensor_copy(out=o_t[64:128, :], in_=O2[0:64, 512:1024])
                # softmax denominators: row 64 of O2 -> f16 SBUF row;
                # partition-broadcast happens later via a K=1 ones-matmul in
                # mk_y_recip -- no DRAM round trip.  DVE, not Act: Act is the
                # scarce engine in the diagonal/tail region (exp backlog).
                row = rot.tile([1, 1024], F16, tag="srow", name="srow", bufs=4)
                nc.vector.tensor_copy(out=row, in_=O2[64:65, :])
                state[("srow", it, p)] = row
                state[("osb", it, p)] = o_t
            return job

        def nfree(it, jb):
            ko = jb - 4 * it
            return 512 - 128 * ko if ko > 0 else 512

        def attn_jobs(it):
            njb = 4 * it + 4
            jobs = []
            for p in range(4):
                for jb in range(njb):
                    c = nfree(it, jb) + (2 * nfree(it, jb - 2) if jb >= 2
                                         else 0)
                    jobs.append((mk_jb(it, p, jb, njb), c))
                jobs.append((mk_tail(it, p, njb),
                             2 * nfree(it, njb - 2) + 2 * nfree(it, njb - 1)))
            return jobs

        # ---------- normalization + output projection ----------

        def mk_y_recip(it, p):
            def job():
                row = state.pop(("srow", it, p))
                ps = psp.tile([P, 512], F32, tag="psA", name="psA", bufs=2)
                # broadcast sums across partitions: ones[1,64]^T @ row[1,512]
                nc.tensor.matmul(ps[0:64, :], ones64, row[0:1, 0:512],
                                 start=True, stop=True)
                nc.tensor.matmul(ps[64:128, :], ones64, row[0:1, 512:1024],
                                 start=True, stop=True)
                rb_t = rot.tile([P, 512], F32, tag=f"rb{p}", name=f"rb{p}",
                                bufs=1)
                nc.vector.reciprocal_approx_fast(out=rb_t, in_=ps)
                state[("rb", it, p)] = rb_t
            return job

        def mk_y_mul(it, p):
            def job():
                g_t = state.pop(("g", it, p))
                o_t = state.pop(("osb", it, p))
                rb_t = state.pop(("rb", it, p))
                tt = rot.tile([P, 512], F32, tag="tt", name="tt", bufs=2)
                # (1 + tanh) * O  -- the 0.5 of the sigmoid identity is folded
                # into wp (host-scaled), the row-sum recip comes via rb.
                nc.vector.scalar_tensor_tensor(
                    out=tt, in0=g_t, scalar=1.0, in1=o_t,
                    op0=mybir.AluOpType.add, op1=mybir.AluOpType.mult)
                y_t = rot.tile([P, 512], F16, tag=f"y{p}", name=f"y{p}", bufs=2)
                nc.vector.tensor_mul(y_t, tt, rb_t)
                state[("y", it, p)] = y_t
            return job

        def norm_jobs(it, ps=range(4)):
            jobs = []
            for p in ps:
                jobs.append((mk_y_recip(it, p), 1024))
                jobs.append((mk_y_mul(it, p), 0))
            return jobs

        def mk_proj(it, m):
            def job():
                ps = psp.tile([P, 512], F32, tag="psA", name="psA", bufs=2)
                for k in range(4):
                    nc.tensor.matmul(ps, wpsb[k][:, m * P:(m + 1) * P],
                                     state[("y", it, k)],
                                     start=(k == 0), stop=(k == 3))
                ob = rot.tile([P, 512], F16, tag="ob", name="ob",
                               bufs=2)
                if it == NTB - 1 and m % 2 == 0:
                    # alternate Act/DVE at the kernel tail so consecutive
                    # drains overlap instead of queueing on one engine
                    nc.scalar.activation(ob, ps,
                                         mybir.ActivationFunctionType.Copy,
                                         bias=0.0, scale=1.0)
                else:
                    nc.vector.tensor_copy(out=ob, in_=ps)
                nc.sync.dma_start(
                    out=outT[m * P:(m + 1) * P, it * 512:(it + 1) * 512],
                    in_=ob)
                if m == 7:
                    for k in range(4):
                        state.pop(("y", it, k))
            return job

        def proj_jobs(it):
            return [(mk_proj(it, m), 2048) for m in range(8)]

        # ---------- emission schedule ----------
        # Pace secondary jobs against primary by PE-work cost, not count, so
        # the cheap diagonal/tail stretch of each i-tile gets proportionally
        # more filler and the PE never starves there.
        def emit_interleaved(primary, secondary):
            ptot = sum(c for _, c in primary) or 1
            stot = sum(c for _, c in secondary) or 1
            si = 0
            pacc = 0.0
            sacc = 0.0
            for job, c in primary:
                job()
                pacc += c
                target = pacc / ptot
                while si < len(secondary) and \
                        (sacc + secondary[si][1] * 0.5) / stot <= target:
                    secondary[si][0]()
                    sacc += secondary[si][1]
                    si += 1
            while si < len(secondary):
                secondary[si][0]()
                si += 1

        # Only the q/kT projections a head-pair actually reads gate its
        # attention: p0 needs just m=0/m=4, p1 m=1/m=5, etc.  Emit m0/m4/m1/m5
        # up front and pace the rest into it0 so the Act exp chain starts
        # ~14us earlier.  v/gate jobs become it0 filler.  gate(tb) is only
        # consumed by norm(tb) one i-tile later, so it shifts one span later
        # too -- this drains surplus PE filler from the early (PE-bound)
        # spans.  gate(3) stays in it2: its tanh would land on it3's
        # critical Act chain.
        for m in (0, 4, 1, 5):
            mk_qk(0, m)()
        for it in range(NTB):
            primary = attn_jobs(it)
            secondary = []
            if it == 0:
                secondary.append((mk_wvg_dma(), 0))
                secondary += v_jobs(0, cost=0)
            if it + 1 < NTB:
                secondary.append((mk_xt_dma(it + 1), 0))
            if it == 0:
                secondary.append((mk_wp_dma(), 0))
                secondary += [(mk_qk(0, m), 4096) for m in (2, 6, 3, 7)]
                secondary += gate_jobs(0)
            if it >= 1:
                secondary += norm_jobs(it - 1)
            if it in (1, 2):
                secondary += gate_jobs(it)
            if it == 2:
                secondary += gate_jobs(3)
            if it == 1:
                secondary += proj_jobs(0)
            if it == NTB - 1:
                # back-load the PE-heavy projection jobs into the final,
                # Act-bound attention segment to fill PE idle slots; the
                # final i-tile's own y-jobs go last so each lands right
                # after its p-group's drain
                secondary += proj_jobs(1) + proj_jobs(2)
                secondary += norm_jobs(3, ps=range(3))
            if it + 1 < NTB:
                secondary += qk_jobs(it + 1)
                secondary += v_jobs(it + 1)
            emit_interleaved(primary, secondary)
        for job, _ in norm_jobs(NTB - 1, ps=[3]) + proj_jobs(NTB - 1):
            job()

    nc.compile()
    return nc


def kernel(x, w_attn, w_proj, w_gate, b_gate):
    x = np.ascontiguousarray(np.asarray(x, dtype=np.float32))
    w_attn = np.asarray(w_attn, dtype=np.float32)
    w_proj = np.asarray(w_proj, dtype=np.float32)
    w_gate = np.asarray(w_gate, dtype=np.float32)
    b_gate = np.asarray(b_gate, dtype=np.float32)

    if "nc" not in _NC_CACHE:
        _NC_CACHE["nc"] = _build_nc()
    nc = _NC_CACHE["nc"]

    in_maps = []
    for c in range(8):
        b, g = c // 2, c % 2
        hsl = slice(g * GC, (g + 1) * GC)
        in_maps.append({
            "xT": np.ascontiguousarray(x[b].T.astype(np.float16)),
            "wqk": np.ascontiguousarray(
                np.concatenate([w_attn[:, hsl],
                                w_attn[:, C + g * GC:C + (g + 1) * GC]],
                               axis=1).astype(np.float16)),
            "wv": np.ascontiguousarray(
                w_attn[:, 2 * C + g * GC:2 * C + (g + 1) * GC].astype(np.float16)),
            "wg": np.ascontiguousarray(w_gate[:, hsl].astype(np.float16)),
            "bgh": np.ascontiguousarray(b_gate[hsl] * 0.5),
            "wp": np.ascontiguousarray((w_proj[hsl, :] * 0.5).astype(np.float16)),
        })

    res = bass_utils.run_bass_kernel_spmd(nc, in_maps, core_ids=list(range(8)))

    out = np.empty((B, T, C), dtype=np.float32)
    for b in range(B):
        acc = res.results[2 * b]["outT"].astype(np.float32)
        acc = acc + res.results[2 * b + 1]["outT"].astype(np.float32)
        out[b] = acc.T
    return out



# revision 36
# speedup vs baseline: 1.1403x; 1.0341x over previous
"""Trainium2 Bass kernel for nn_CausalSelfAttention_14980845928591.

Full-input contract: kernel(**inputs) takes the unsharded numpy inputs and
returns the full [B, T, C] output. Internally shards across 8 NeuronCores:
data-parallel over B (4 batches) x tensor-parallel over heads (2 groups of 8
heads). The output projection is a partial sum over head groups, reduced on
the host.

Single merged pipeline per core (no phase barriers):
  - projections (q/k/v/gate) per 512-wide T-block in fp32r (K=128 matmuls
    stream fastest in fp32r; weight load hides internally),
  - causal attention per i-tile in fp16 (K=64 / M=65 matmuls hit full PE
    stream rate in 16-bit; fp16 keeps quantization error ~1e-3),
  - output projection in fp32r.
Emission interleaves attention jb-steps (primary) with projection tile-jobs
(secondary) so the PE queue always holds independent work behind the
exp-dependent AV matmuls. exp runs on Act; sigmoid is computed as
0.5*(1+tanh(x/2)) so Act only ever needs the exp/tanh table (no reloads);
the 0.5 is folded into host-scaled w_proj and the +1 into a fused
scalar_tensor_tensor on DVE. Softmax row sums ride as a 65th V row; the
reciprocal uses the fast DVE approximation and is broadcast across
partitions via a DRAM round trip, all software-pipelined one i-tile behind
attention.
"""
import sys

sys.path.insert(0, "/opt/trn_rl_repo")

import numpy as np

import concourse.bass as bass
import concourse.mybir as mybir
import concourse.tile as tile
from concourse import bacc, bass_utils

# Problem shapes (hardcoded per contract).
B, T, C = 4, 2048, 1024
H, D = 16, 64
HL = 8            # heads per core
GC = HL * D       # 512: local q/k/v/gate/proj-row columns
P = 128
KC = C // P       # 8 contraction chunks
NTB = T // 512    # 4 T-blocks of 512
F32 = mybir.dt.float32
F32R = mybir.dt.float32r
F16 = mybir.dt.float16

_NC_CACHE = {}


def _build_nc():
    nc = bacc.Bacc("TRN2", target_bir_lowering=False, debug=False, num_devices=8)

    xT = nc.dram_tensor("xT", [C, T], F16, kind="ExternalInput")
    wqk = nc.dram_tensor("wqk", [C, 2 * GC], F16, kind="ExternalInput")
    wv = nc.dram_tensor("wv", [C, GC], F16, kind="ExternalInput")
    wg = nc.dram_tensor("wg", [C, GC], F16, kind="ExternalInput")
    bgh = nc.dram_tensor("bgh", [GC], F32, kind="ExternalInput")   # 0.5*b_gate
    wp = nc.dram_tensor("wp", [GC, C], F16, kind="ExternalInput")  # 0.5*w_proj
    outT = nc.dram_tensor("outT", [C, T], F16, kind="ExternalOutput")

    EXP = mybir.ActivationFunctionType.Exp
    TANH = mybir.ActivationFunctionType.Tanh

    with tile.TileContext(nc) as tc, \
         tc.tile_pool(name="pers", bufs=1) as pers, \
         tc.tile_pool(name="rot", bufs=2) as rot, \
         tc.tile_pool(name="psp", bufs=1, space="PSUM") as psp:

        # ---------- persistent tiles ----------
        kT = [pers.tile([P, T], F16, tag=f"kT{i}", name=f"kT{i}") for i in range(4)]
        vsb = [pers.tile([P, HL, 65], F16, tag=f"v{j}", name=f"v{j}")
               for j in range(T // P)]
        wqksb = [pers.tile([P, 2 * GC], F16, tag=f"wqk{k}", name=f"wqk{k}")
                 for k in range(KC)]
        wvsb = [pers.tile([P, GC], F16, tag=f"wv{k}", name=f"wv{k}")
                for k in range(KC)]
        wgsb = [pers.tile([P, GC], F16, tag=f"wg{k}", name=f"wg{k}")
                for k in range(KC)]
        wpsb = [pers.tile([P, C], F16, tag=f"wp{k}", name=f"wp{k}")
                for k in range(4)]
        umaskf = pers.tile([P, P], F32, tag="umaskf")
        umask = pers.tile([P, P], F16, tag="umask")
        zbias = pers.tile([P, 1], F32, tag="zbias")
        bgsb = pers.tile([P, 4], F32, tag="bg")
        ones64 = pers.tile([1, 64], F16, tag="ones64")
        warm = pers.tile([P, 256], F16, tag="warm")

        # constants
        nc.vector.memset(warm, 0.0)
        nc.vector.memset(ones64, 1.0)
        nc.gpsimd.memset(zbias, 0.0)
        nc.gpsimd.memset(umaskf, 1.0)
        # keep element where (col - row) >= 0 (upper triangular incl diag)
        nc.gpsimd.affine_select(
            out=umaskf, in_=umaskf, compare_op=mybir.AluOpType.is_ge,
            fill=0.0, base=0, pattern=[[1, P]], channel_multiplier=-1,
        )
        nc.vector.tensor_copy(out=umask, in_=umaskf)
        nc.sync.dma_start(out=bgsb, in_=bgh.rearrange("(m p) -> p m", m=4))

        state = {}

        # PE warmup: dummy matmuls spanning the initial DMA window so the
        # HAM clock gate flips to 8/8 before the first real matmul and the
        # projections never run cold.  Only depends on a vector memset, so
        # the chain starts within ~1us of kernel entry.
        wps = psp.tile([P, 512], F32, tag="psA", name="psA", bufs=2)
        for _ in range(28):
            nc.tensor.matmul(wps[:, 0:256], warm[:, 0:P], warm,
                             start=True, stop=True)

        # ---------- startup: wqk + x(tb0) interleaved ----------
        xt0 = []
        for k in range(KC):
            nc.sync.dma_start(out=wqksb[k], in_=wqk[k * P:(k + 1) * P, :])
            t_ = rot.tile([P, 512], F16, tag="xs", name="xs", bufs=12)
            nc.sync.dma_start(out=t_, in_=xT[k * P:(k + 1) * P, 0:512])
            xt0.append(t_)
        state[("xt", 0)] = xt0

        # ---------- job builders ----------
        def mk_xt_dma(tb):
            def job():
                xt = []
                for k in range(KC):
                    t_ = rot.tile([P, 512], F16, tag="xs", name="xs", bufs=12)
                    nc.sync.dma_start(
                        out=t_, in_=xT[k * P:(k + 1) * P, tb * 512:(tb + 1) * 512])
                    xt.append(t_)
                state[("xt", tb)] = xt
            return job

        def mk_wvg_dma():
            def job():
                for k in range(KC):
                    nc.sync.dma_start(out=wvsb[k], in_=wv[k * P:(k + 1) * P, :])
                    nc.sync.dma_start(out=wgsb[k], in_=wg[k * P:(k + 1) * P, :])
            return job

        def mk_wp_dma():
            def job():
                for k in range(4):
                    nc.sync.dma_start(out=wpsb[k], in_=wp[k * P:(k + 1) * P, :])
            return job

        def mk_qk(tb, m):
            def job():
                xt = state[("xt", tb)]
                tsl = slice(tb * 512, (tb + 1) * 512)
                ps = psp.tile([P, 512], F32, tag="psA", name="psA", bufs=2)
                for k in range(KC):
                    nc.tensor.matmul(ps, wqksb[k][:, m * P:(m + 1) * P], xt[k],
                                     start=(k == 0), stop=(k == KC - 1))
                if m < 4:
                    q_t = rot.tile([P, 512], F16, tag=f"q{m}", name=f"q{m}", bufs=2)
                    nc.vector.tensor_copy(out=q_t, in_=ps)
                    state[("q", tb, m)] = q_t
                else:
                    nc.vector.tensor_copy(out=kT[m - 4][:, tsl], in_=ps)
            return job

        def mk_gate(tb, m):
            def job():
                xt = state[("xt", tb)]
                ps = psp.tile([P, 512], F32, tag="psA", name="psA", bufs=2)
                for k in range(KC):
                    nc.tensor.matmul(ps, wgsb[k][:, m * P:(m + 1) * P], xt[k],
                                     start=(k == 0), stop=(k == KC - 1))
                g_t = rot.tile([P, 512], F16, tag=f"g{m}", name=f"g{m}", bufs=2)
                # tanh(0.5*x + 0.5*b); sigmoid = 0.5*(1+tanh(...)): the +1 is
                # fused into the gate multiply, the 0.5 into host-scaled wp.
                nc.scalar.activation(g_t, ps, TANH, bias=bgsb[:, m:m + 1],
                                     scale=0.5)
                state[("g", tb, m)] = g_t
            return job

        def mk_v(tb, mt):
            def job():
                xt = state[("xt", tb)]
                j = tb * 4 + mt
                ps = psp.tile([P, 512], F32, tag="psA", name="psA", bufs=2)
                for k in range(KC):
                    nc.tensor.matmul(ps, xt[k][:, mt * P:(mt + 1) * P], wvsb[k],
                                     start=(k == 0), stop=(k == KC - 1))
                nc.vector.tensor_copy(
                    out=vsb[j][:, :, 0:64],
                    in_=ps.rearrange("p (h d) -> p h d", h=HL))
                nc.vector.memset(vsb[j][:, :, 64:65], 1.0)
            return job

        def qk_jobs(tb):
            return [(mk_qk(tb, m), 4096) for m in range(8)]

        def v_jobs(tb, cost=4096):
            return [(mk_v(tb, mt), cost) for mt in range(4)]

        def gate_jobs(tb):
            return [(mk_gate(tb, m), 4096) for m in range(4)]

        # ---------- attention ----------
        def emit_av(it, p, jb, njb):
            ko = jb - 4 * it
            c0 = P * ko if ko > 0 else 0
            pT = state.pop(("pT", it, p, jb))
            st, sp = (jb == 0), (jb == njb - 1)
            if st:
                state[("O2", it, p)] = psp.tile([65, 1024], F32, tag="O2",
                                                name="O2", bufs=1)
            O2 = state[("O2", it, p)]
            nc.tensor.matmul(O2[:, c0:512], vsb[jb][:, 2 * p, :],
                             pT[:, c0:512], start=st, stop=sp,
                             skip_group_check=True)
            nc.tensor.matmul(O2[:, 512 + c0:1024], vsb[jb][:, 2 * p + 1, :],
                             pT[:, 512 + c0:1024], start=st, stop=sp,
                             skip_group_check=True)

        def mk_jb(it, p, jb, njb):
            def job():
                ko = jb - 4 * it
                c0 = P * ko if ko > 0 else 0
                jsl = slice(jb * P, (jb + 1) * P)
                q_t = state[("q", it, p)]
                s2 = psp.tile([P, 1024], F32, tag="s2", name="s2", bufs=2)
                nc.tensor.matmul(s2[:, c0:512], kT[p][0:64, jsl],
                                 q_t[0:64, c0:512], start=True, stop=True)
                nc.tensor.matmul(s2[:, 512 + c0:1024], kT[p][64:128, jsl],
                                 q_t[64:128, c0:512], start=True, stop=True)
                pT = rot.tile([P, 1024], F16, tag="pT", name="pT", bufs=3)
                if ko > 0:
                    # one strided activation covers both heads' live columns
                    sv = s2.rearrange("p (h f) -> p h f", h=2)[:, :, c0:512]
                    pv = pT.rearrange("p (h f) -> p h f", h=2)[:, :, c0:512]
                    nc.scalar.activation(pv, sv, EXP, bias=zbias, scale=0.125)
                else:
                    nc.scalar.activation(pT, s2, EXP, bias=zbias, scale=0.125)
                if ko >= 0:
                    # causal mask on the diagonal 128x128 block of both heads
                    # via one gpsimd affine_select (keep where col-row >= 0);
                    # gpsimd is otherwise idle, freeing DVE for drains/casts.
                    mv = pT.rearrange("p (h f) -> p h f", h=2)[:, :, c0:c0 + P]
                    nc.gpsimd.affine_select(
                        out=mv, in_=mv, compare_op=mybir.AluOpType.is_ge,
                        fill=0.0, base=0, pattern=[[0, 2], [1, P]],
                        channel_multiplier=-1,
                    )
                state[("pT", it, p, jb)] = pT
                # AV trails QK by two jb-steps: by the time the AV pair
                # reaches the head of the PE FIFO its exp+mask inputs are
                # long done, so the PE never head-of-line blocks on Act.
                if jb > 1:
                    emit_av(it, p, jb - 2, njb)
            return job

        def mk_tail(it, p, njb):
            def job():
                if njb > 1:
                    emit_av(it, p, njb - 2, njb)
                emit_av(it, p, njb - 1, njb)
                O2 = state.pop(("O2", it, p))
                o_t = rot.tile([P, 512], F16, tag=f"o{p}", name=f"o{p}", bufs=2)
                nc.vector.tensor_copy(out=o_t[0:64, :], in_=O2[0:64, 0:512])
                nc.vector.tensor_copy(out=o_t[64:128, :], in_=O2[0:64, 512:1024])
                # softmax denominators: row 64 of O2 -> f16 SBUF row;
                # partition-broadcast happens later via a K=1 ones-matmul in
                # mk_y_recip -- no DRAM round trip.  DVE, not Act: Act is the
                # scarce engine in the diagonal/tail region (exp backlog).
                row = rot.tile([1, 1024], F16, tag="srow", name="srow", bufs=4)
                nc.vector.tensor_copy(out=row, in_=O2[64:65, :])
                state[("srow", it, p)] = row
                state[("osb", it, p)] = o_t
            return job

        def nfree(it, jb):
            ko = jb - 4 * it
            return 512 - 128 * ko if ko > 0 else 512

        def attn_jobs(it):
            # Job costs model wall TIME (in 2.4GHz PE cycles), not just PE
            # work: a jb step is Act-bound (exp is (2*nf+352)cyc at 1.2GHz),
            # so pacing by max(PE, Act) feeds proportionally more filler into
            # the Act-bound stretches and keeps the PE warm at i-tile ends.
            njb = 4 * it + 4
            jobs = []
            for p in range(4):
                for jb in range(njb):
                    pe = nfree(it, jb) + (2 * nfree(it, jb - 2) if jb >= 2
                                          else 0)
                    act = (2 * nfree(it, jb) + 352) * 2
                    jobs.append((mk_jb(it, p, jb, njb), max(pe, act)))
                tail_pe = 2 * nfree(it, njb - 2) + 2 * nfree(it, njb - 1)
                jobs.append((mk_tail(it, p, njb), tail_pe + 2400))
            return jobs

        # ---------- normalization + output projection ----------

        def mk_y_recip(it, p):
            def job():
                row = state.pop(("srow", it, p))
                ps = psp.tile([P, 512], F32, tag="psA", name="psA", bufs=2)
                # broadcast sums across partitions: ones[1,64]^T @ row[1,512]
                nc.tensor.matmul(ps[0:64, :], ones64, row[0:1, 0:512],
                                 start=True, stop=True)
                nc.tensor.matmul(ps[64:128, :], ones64, row[0:1, 512:1024],
                                 start=True, stop=True)
                rb_t = rot.tile([P, 512], F32, tag=f"rb{p}", name=f"rb{p}",
                                bufs=1)
                nc.vector.reciprocal_approx_fast(out=rb_t, in_=ps)
                state[("rb", it, p)] = rb_t
            return job

        def mk_y_mul(it, p):
            def job():
                g_t = state.pop(("g", it, p))
                o_t = state.pop(("osb", it, p))
                rb_t = state.pop(("rb", it, p))
                tt = rot.tile([P, 512], F32, tag="tt", name="tt", bufs=2)
                # (1 + tanh) * O  -- the 0.5 of the sigmoid identity is folded
                # into wp (host-scaled), the row-sum recip comes via rb.
                nc.vector.scalar_tensor_tensor(
                    out=tt, in0=g_t, scalar=1.0, in1=o_t,
                    op0=mybir.AluOpType.add, op1=mybir.AluOpType.mult)
                y_t = rot.tile([P, 512], F16, tag=f"y{p}", name=f"y{p}", bufs=2)
                nc.vector.tensor_mul(y_t, tt, rb_t)
                state[("y", it, p)] = y_t
            return job

        def norm_jobs(it, ps=range(4)):
            jobs = []
            for p in ps:
                jobs.append((mk_y_recip(it, p), 1024))
                jobs.append((mk_y_mul(it, p), 0))
            return jobs

        def mk_proj(it, m):
            def job():
                ps = psp.tile([P, 512], F32, tag="psA", name="psA", bufs=2)
                for k in range(4):
                    nc.tensor.matmul(ps, wpsb[k][:, m * P:(m + 1) * P],
                                     state[("y", it, k)],
                                     start=(k == 0), stop=(k == 3))
                ob = rot.tile([P, 512], F16, tag="ob", name="ob",
                               bufs=2)
                if it == NTB - 1 and m % 2 == 0:
                    # alternate Act/DVE at the kernel tail so consecutive
                    # drains overlap instead of queueing on one engine
                    nc.scalar.activation(ob, ps,
                                         mybir.ActivationFunctionType.Copy,
                                         bias=0.0, scale=1.0)
                else:
                    nc.vector.tensor_copy(out=ob, in_=ps)
                nc.sync.dma_start(
                    out=outT[m * P:(m + 1) * P, it * 512:(it + 1) * 512],
                    in_=ob)
                if m == 7:
                    for k in range(4):
                        state.pop(("y", it, k))
            return job

        def proj_jobs(it):
            return [(mk_proj(it, m), 2048) for m in range(8)]

        # ---------- emission schedule ----------
        # Pace secondary jobs against primary by PE-work cost, not count, so
        # the cheap diagonal/tail stretch of each i-tile gets proportionally
        # more filler and the PE never starves there.
        def emit_interleaved(primary, secondary):
            ptot = sum(c for _, c in primary) or 1
            stot = sum(c for _, c in secondary) or 1
            si = 0
            pacc = 0.0
            sacc = 0.0
            for job, c in primary:
                job()
                pacc += c
                target = pacc / ptot
                while si < len(secondary) and \
                        (sacc + secondary[si][1] * 0.5) / stot <= target:
                    secondary[si][0]()
                    sacc += secondary[si][1]
                    si += 1
            while si < len(secondary):
                secondary[si][0]()
                si += 1

        # Only tb0's q/k projections gate the start of attention; v/gate
        # jobs become it0 filler.  gate(tb) is only consumed by norm(tb)
        # one i-tile later, so it shifts one span later too -- this drains
        # surplus PE filler from the early (PE-bound) spans.  gate(3) stays
        # in it2: its tanh would land on it3's critical Act chain.
        for job, _ in qk_jobs(0):
            job()
        for it in range(NTB):
            primary = attn_jobs(it)
            secondary = []
            if it == 0:
                secondary.append((mk_wvg_dma(), 0))
                secondary += v_jobs(0, cost=0)
            if it + 1 < NTB:
                secondary.append((mk_xt_dma(it + 1), 0))
            if it == 0:
                secondary.append((mk_wp_dma(), 0))
                secondary += gate_jobs(0)
            if it >= 1:
                secondary += norm_jobs(it - 1)
            if it in (1, 2):
                secondary += gate_jobs(it)
            if it == 2:
                secondary += gate_jobs(3)
            # qk/v(it+1) have a deadline (start of it+1) -- pace them before
            # the deadline-free proj jobs so the end-of-span flush is small.
            if it + 1 < NTB:
                secondary += qk_jobs(it + 1)
                secondary += v_jobs(it + 1)
            if it == 1:
                secondary += proj_jobs(0)
            if it == NTB - 1:
                # back-load the PE-heavy projection jobs into the final,
                # Act-bound attention segment to fill PE idle slots; the
                # final i-tile's own y-jobs go last so each lands right
                # after its p-group's drain
                secondary += proj_jobs(1) + proj_jobs(2)
                secondary += norm_jobs(3, ps=range(3))
            emit_interleaved(primary, secondary)
        for job, _ in norm_jobs(NTB - 1, ps=[3]) + proj_jobs(NTB - 1):
            job()

    nc.compile()
    return nc


def kernel(x, w_attn, w_proj, w_gate, b_gate):
    x = np.ascontiguousarray(np.asarray(x, dtype=np.float32))
    w_attn = np.asarray(w_attn, dtype=np.float32)
    w_proj = np.asarray(w_proj, dtype=np.float32)
    w_gate = np.asarray(w_gate, dtype=np.float32)
    b_gate = np.asarray(b_gate, dtype=np.float32)

    if "nc" not in _NC_CACHE:
        _NC_CACHE["nc"] = _build_nc()
    nc = _NC_CACHE["nc"]

    in_maps = []
    for c in range(8):
        b, g = c // 2, c % 2
        hsl = slice(g * GC, (g + 1) * GC)
        in_maps.append({
            "xT": np.ascontiguousarray(x[b].T.astype(np.float16)),
            "wqk": np.ascontiguousarray(
                np.concatenate([w_attn[:, hsl],
                                w_attn[:, C + g * GC:C + (g + 1) * GC]],
                               axis=1).astype(np.float16)),
            "wv": np.ascontiguousarray(
                w_attn[:, 2 * C + g * GC:2 * C + (g + 1) * GC].astype(np.float16)),
            "wg": np.ascontiguousarray(w_gate[:, hsl].astype(np.float16)),
            "bgh": np.ascontiguousarray(b_gate[hsl] * 0.5),
            "wp": np.ascontiguousarray((w_proj[hsl, :] * 0.5).astype(np.float16)),
        })

    res = bass_utils.run_bass_kernel_spmd(nc, in_maps, core_ids=list(range(8)))

    out = np.empty((B, T, C), dtype=np.float32)
    for b in range(B):
        acc = res.results[2 * b]["outT"].astype(np.float32)
        acc = acc + res.results[2 * b + 1]["outT"].astype(np.float32)
        out[b] = acc.T
    return out

